# revision 22
# baseline (speedup 1.0000x reference)
"""Multi-head attention (B=4, S=2048, D=1024, H=16) on 8 Trainium2 cores.

Sharding: core = (batch b, head-group g): 4 batches x 2 groups of 8 heads.

Per core (feature-major layouts):
  KT/QT = (x @ W^T)^T        [512, S] bf16
  V'    = x @ Wv^T           [S, 8 heads x (64 V cols + ones col)] bf16
  attention in q-chunks of 1024 (bf16 moving-operand max), per head:
    per kt tile: scores^T[k, q] = 1 MM (contraction 64, N=1024),
    exp on ACT (scale 1/8, bias -2) -> et bf16 [128, 1024],
    attnV MM (M=65, N=1024) accumulating u[0:65]; row 64 = softmax
    denominator via the V' ones column.
  denominators DMA-restacked into [8, 1024]; one batched DVE reciprocal
  per q-chunk; per-head broadcast via ones-matmul; in-place normalize of
  attnT; O-projection (bf16) -> outT partials.
Host: per batch, sum the two groups' outT partials, transpose, add b_o.

Softmax skips max-subtraction (scores ~N(0,1)); exp biased by -2
(shift-invariant, keeps exp outputs in a comfortable range).

The whole kernel is software-pipelined around the ACT-bound attention
loop: K/V/Q/O-projection matmuls and the previous chunk's normalize are
emitted as deadline-scheduled fillers inside the attention slots, so the
PE never head-of-line blocks on ACT output.
"""

import collections

import ml_dtypes
import numpy as np

import concourse.bass as bass
import concourse.mybir as mybir
import concourse.tile as tile
from concourse import bacc

B = 4
S = 2048
D = 1024
H = 16
DK = 64
NCORES = 8
GROUPS = 2
HPC = H // GROUPS  # heads per core (8)
FC = HPC * DK  # local features per core (512)
P = 128

F32 = mybir.dt.float32
BF16 = mybir.dt.bfloat16

VW = 128  # per-head V' width: 64 V + ones col + pad (FWL wants 128)
EXP_BIAS = -2.0

_NC_CACHE = {}


def build_nc(bias=False):
    s, d, fc, hpc = S, D, FC, HPC
    ndt = d // P  # 8 contraction tiles for projections
    nft = fc // P  # 4 local feature tiles
    nq = 512  # attention q-chunk width
    nqc = s // nq  # 4 q chunks
    nkt = s // P  # 16 k tiles
    npair = nkt // 2  # 8 kt-pair slots per head
    inv_sqrt_dk = 1.0 / float(np.sqrt(DK))
    nslots = hpc * npair  # 64 (head, kt-pair) slots per q-chunk

    nc = bacc.Bacc("TRN2", target_bir_lowering=False, debug=False)

    xqT = nc.dram_tensor("xqT", [d, s], BF16, kind="ExternalInput").ap()
    xkT = nc.dram_tensor("xkT", [d, s], BF16, kind="ExternalInput").ap()
    xvT = nc.dram_tensor("xvT", [d, s], BF16, kind="ExternalInput").ap()
    wqT = nc.dram_tensor("wqT", [d, fc], BF16, kind="ExternalInput").ap()
    wkT = nc.dram_tensor("wkT", [d, fc], BF16, kind="ExternalInput").ap()
    wvT = nc.dram_tensor("wvT", [d, fc], BF16, kind="ExternalInput").ap()
    woT = nc.dram_tensor("woT", [fc, d], BF16, kind="ExternalInput").ap()
    outT = nc.dram_tensor("outT", [d, s], F32, kind="ExternalOutput").ap()
    if bias:
        bq = nc.dram_tensor("bq", [1, fc], BF16, kind="ExternalInput").ap()
        bk = nc.dram_tensor("bk", [1, fc], BF16, kind="ExternalInput").ap()
        bv = nc.dram_tensor("bv", [1, fc], BF16, kind="ExternalInput").ap()

    ones8_dram = nc.inline_tensor(
        np.ones((8, 64), ml_dtypes.bfloat16), name="ones8_c"
    ).ap()
    onesv_dram = nc.inline_tensor(
        np.ones((P, hpc), ml_dtypes.bfloat16), name="onesv_c"
    ).ap()
    biasA_dram = nc.inline_tensor(
        np.full((P, 1), EXP_BIAS, np.float32), name="biasA_c"
    ).ap()
    ones1_dram = nc.inline_tensor(
        np.ones((1, 512), ml_dtypes.bfloat16), name="ones1_c"
    ).ap()

    with tile.TileContext(nc) as tc:
        with (
            tc.tile_pool(name="sb", bufs=1) as sb,
            tc.tile_pool(name="ps", bufs=1, space="PSUM") as ps,
        ):
            kt_t = sb.tile([P, nft, s], BF16, tag="KT")
            qt_t = sb.tile([P, nft, s], BF16, tag="QT")
            attnT = qt_t  # attnT(h, qc) overwrites QT columns already consumed
            xk_t = sb.tile([P, ndt, s], BF16, tag="xk")
            xv_t = sb.tile([P, ndt, s], BF16, tag="xv")
            vp_t = sb.tile([P, nkt, hpc, VW], BF16, tag="Vp")
            wk_t = sb.tile([P, ndt, fc], BF16, tag="wk")
            wq_t = sb.tile([P, ndt, fc], BF16, tag="wq")
            wv_t = sb.tile([P, ndt, fc], BF16, tag="wv")
            wo_t = sb.tile([P, fc // P, d], BF16, tag="wo")
            den_a = sb.tile([4, nq], BF16, tag="den_a")
            den_b = sb.tile([4, nq], BF16, tag="den_b")
            rc_a = sb.tile([4, nq], BF16, tag="rc_a")
            rc_b = sb.tile([4, nq], BF16, tag="rc_b")
            rcrow = sb.tile([1, hpc * nq], BF16, tag="rcrow")
            ones8 = sb.tile([8, 64], BF16, tag="ones8")
            onesv = sb.tile([P, hpc], BF16, tag="onesv")
            biasA = sb.tile([P, 1], F32, tag="biasA")
            warm = sb.tile([8, 64], BF16, tag="warm")

            def dma_split(dst, src_ap, n):
                # split big loads across DMA queues; alternate issuing engine
                for i in range(n):
                    eng = nc.sync if i % 2 == 0 else nc.gpsimd
                    eng.dma_start(out=dst[:, i], in_=src_ap[:, i])

            nc.sync.dma_start(out=ones8[:], in_=ones8_dram)
            nc.gpsimd.dma_start(out=onesv[:], in_=onesv_dram)
            nc.sync.dma_start(out=biasA[:], in_=biasA_dram)
            if bias:
                ones1 = sb.tile([1, 512], BF16, tag="ones1")
                nc.sync.dma_start(out=ones1[:], in_=ones1_dram)
                bq_t = sb.tile([1, fc], BF16, tag="bq")
                bk_t = sb.tile([1, fc], BF16, tag="bk")
                bv_t = sb.tile([1, fc], BF16, tag="bv")
                nc.sync.dma_start(out=bq_t[:], in_=bq)
                nc.sync.dma_start(out=bk_t[:], in_=bk)
                nc.sync.dma_start(out=bv_t[:], in_=bv)

            # warm up the ACT exp table during the initial DMA wait
            nc.scalar.activation(
                warm[:],
                ones8[:],
                mybir.ActivationFunctionType.Exp,
                scale=inv_sqrt_dk,
                bias=biasA[0:8, :],
            )

            # weights/x loads with COARSE DMAs (each dma_start costs ~600ns
            # of issuing-engine time; keep the instruction count low).
            # First-score gate: wk + xk(sc0) + wq + xq0.
            wk_r = wkT.rearrange("(t p) f -> p t f", p=P)
            wq_r = wqT.rearrange("(t p) f -> p t f", p=P)
            xk_r = xkT.rearrange("(t p) (c n) -> p t c n", p=P, n=512)
            x0_t = sb.tile([P, ndt, nq], BF16, tag="xq", bufs=1)
            xq_r0 = xqT[:, 0:nq].rearrange("(t p) s -> p t s", p=P)
            # gate: wk+xk(sc0) for K-proj, wq+x0 for Q-proj. ~1.3MB/queue.
            nc.sync.dma_start(out=wk_t[:, 0:4], in_=wk_r[:, 0:4])
            nc.gpsimd.dma_start(out=wk_t[:, 4:8], in_=wk_r[:, 4:8])
            nc.scalar.dma_start(out=wq_t[:, 0:8], in_=wq_r[:, 0:8])
            nc.sync.dma_start(out=xk_t[:, 0:4, 0:512], in_=xk_r[:, 0:4, 0])
            nc.gpsimd.dma_start(out=xk_t[:, 4:8, 0:512], in_=xk_r[:, 4:8, 0])
            nc.scalar.dma_start(out=x0_t[:, 0:3], in_=xq_r0[:, 0:3])
            nc.sync.dma_start(out=x0_t[:, 3:6], in_=xq_r0[:, 3:6])
            nc.gpsimd.dma_start(out=x0_t[:, 6:8], in_=xq_r0[:, 6:8])
            xq_chunks0 = x0_t
            # V-side: resident xv + wv
            wv_r = wvT.rearrange("(t p) f -> p t f", p=P)
            nc.scalar.dma_start(out=wv_t[:, 0:4], in_=wv_r[:, 0:4])
            nc.scalar.dma_start(out=wv_t[:, 4:8], in_=wv_r[:, 4:8])
            xv_r = xvT.rearrange("(t p) s -> p t s", p=P)
            for i in range(4):
                eng = nc.sync if i % 2 == 0 else nc.gpsimd
                eng.dma_start(
                    out=xv_t[:, 2 * i : 2 * i + 2, 0:1024],
                    in_=xv_r[:, 2 * i : 2 * i + 2, 0:1024],
                )
            # rest of xk (sc1 first: needed by pair 2), interleaved with xv
            xk_full = xkT.rearrange("(t p) s -> p t s", p=P)
            for i in range(4):
                eng = nc.sync if i % 2 == 0 else nc.gpsimd
                eng.dma_start(
                    out=xk_t[:, 2 * i : 2 * i + 2, 512:1024],
                    in_=xk_full[:, 2 * i : 2 * i + 2, 512:1024],
                )
            for i in range(4):
                eng = nc.gpsimd if i % 2 == 0 else nc.sync
                eng.dma_start(
                    out=xv_t[:, 2 * i : 2 * i + 2, 1024:2048],
                    in_=xv_r[:, 2 * i : 2 * i + 2, 1024:2048],
                )
            for i in range(4):
                eng = nc.sync if i % 2 == 0 else nc.gpsimd
                eng.dma_start(
                    out=xk_t[:, 2 * i : 2 * i + 2, 1024:2048],
                    in_=xk_full[:, 2 * i : 2 * i + 2, 1024:2048],
                )
            wo_r = woT.rearrange("(t p) j -> p t j", p=P)
            nc.scalar.dma_start(out=wo_t[:, 0:2], in_=wo_r[:, 0:2])
            nc.scalar.dma_start(out=wo_t[:, 2:4], in_=wo_r[:, 2:4])

            # ---------- emission helpers ----------
            def kproj_ft(ft, sc):
                acc = ps.tile([P, 512], F32, tag="acc", bufs=2)
                first = True
                if bias:
                    nc.tensor.matmul(
                        acc[:],
                        lhsT=bk_t[0:1, ft * P : (ft + 1) * P],
                        rhs=ones1[0:1, :],
                        start=True,
                        stop=False,
                    )
                    first = False
                for dt in range(ndt):
                    nc.tensor.matmul(
                        acc[:],
                        lhsT=wk_t[:, dt, ft * P : (ft + 1) * P],
                        rhs=xk_t[:, dt, sc * 512 : (sc + 1) * 512],
                        start=(dt == 0 and first),
                        stop=(dt == ndt - 1),
                    )
                nc.vector.tensor_copy(kt_t[:, ft, sc * 512 : (sc + 1) * 512], acc[:])

            xq_chunks = {}

            def qproj_ft(qc, ft):
                qsl = slice(qc * nq, (qc + 1) * nq)
                if qc not in xq_chunks:
                    x_t = sb.tile([P, ndt, nq], BF16, tag="xq", bufs=1)
                    xq_r = xqT[:, qsl].rearrange("(t p) s -> p t s", p=P)
                    nc.sync.dma_start(out=x_t[:, 0:4], in_=xq_r[:, 0:4])
                    nc.gpsimd.dma_start(out=x_t[:, 4:8], in_=xq_r[:, 4:8])
                    xq_chunks[qc] = x_t
                x_t = xq_chunks[qc]
                acc = ps.tile([P, 512], F32, tag="acc", bufs=2)
                first = True
                if bias:
                    nc.tensor.matmul(
                        acc[:],
                        lhsT=bq_t[0:1, ft * P : (ft + 1) * P],
                        rhs=ones1[0:1, :],
                        start=True,
                        stop=False,
                    )
                    first = False
                for dt in range(ndt):
                    nc.tensor.matmul(
                        acc[:],
                        lhsT=wq_t[:, dt, ft * P : (ft + 1) * P],
                        rhs=x_t[:, dt, :],
                        start=(dt == 0 and first),
                        stop=(dt == ndt - 1),
                    )
                nc.vector.tensor_copy(qt_t[:, ft, qsl], acc[:])

            def vproj_st(st):
                acc = ps.tile([P, hpc, DK], F32, tag="acc", bufs=2)
                first = True
                if bias:
                    nc.tensor.matmul(
                        acc[:, :, :],
                        lhsT=ones1[0:1, 0:P],
                        rhs=bv_t[0:1, :],
                        start=True,
                        stop=False,
                    )
                    first = False
                for dt in range(ndt):
                    nc.tensor.matmul(
                        acc[:, :, :],
                        lhsT=xv_t[:, dt, st * P : (st + 1) * P],
                        rhs=wv_t[:, dt, :],
                        start=(dt == 0 and first),
                        stop=(dt == ndt - 1),
                    )
                nc.vector.tensor_copy(vp_t[:, st, :, 0:DK], acc[:])
                nc.vector.tensor_copy(vp_t[:, st, :, DK], onesv[:])

            ot_pair = {}

            def oproj_jt(qc, jt):
                qsl = slice(qc * nq, (qc + 1) * nq)
                acc = ps.tile([P, 512], F32, tag="acc", bufs=2)
                for ct in range(fc // P):
                    nc.tensor.matmul(
                        acc[:],
                        lhsT=wo_t[:, ct, jt * P : (jt + 1) * P],
                        rhs=attnT[:, ct, qsl],
                        start=(ct == 0),
                        stop=(ct == fc // P - 1),
                    )
                # pair two jt evictions into one store DMA
                if jt % 2 == 0:
                    ot_pair[qc] = sb.tile([P, 2, 512], F32, tag="ot", bufs=2, name="ot2")
                ot = ot_pair[qc]
                nc.vector.tensor_copy(ot[:, jt % 2, :], acc[:])
                if jt % 2 == 1:
                    nc.gpsimd.dma_start(
                        out=outT[(jt - 1) * P : (jt + 1) * P, qsl].rearrange(
                            "(j p) n -> p j n", p=P
                        ),
                        in_=ot[:],
                    )

            def normalize_head(qc, h, dma=True):
                # emitted AFTER the recip covering head h is emitted
                qsl = slice(qc * nq, (qc + 1) * nq)
                tp = h // 2
                hp = (h % 2) * 64
                hsl = slice(hp, hp + 64)
                if dma:
                    rc = rc_a[h : h + 1, :] if h < 4 else rc_b[h - 4 : h - 3, :]
                    nc.gpsimd.dma_start(
                        out=rcrow[0:1, h * nq : (h + 1) * nq], in_=rc
                    )
                pbx = ps.tile([P, nq], F32, tag="pbx", bufs=1)
                nc.tensor.matmul(
                    pbx[hsl, :],
                    lhsT=ones8[0:1, :],
                    rhs=rcrow[0:1, h * nq : (h + 1) * nq],
                    start=True,
                    stop=True,
                )
                nc.vector.tensor_mul(
                    attnT[hsl, tp, qsl], attnT[hsl, tp, qsl], pbx[hsl, :]
                )

            def recip_head(h, dstage):
                # per-head recip for the last chunk (overlaps attention
                # instead of idling the PE tail). Partition-0 tiles only:
                # unaligned partition starts are illegal on the DVE.
                den_solo = sb.tile([1, nq], BF16, tag="dsolo", bufs=2)
                nc.gpsimd.dma_start(out=den_solo[:], in_=dstage[64:65, :])
                rc_solo = sb.tile([1, nq], BF16, tag="rsolo", bufs=2)
                with nc.allow_low_precision(reason="softmax denominator recip"):
                    nc.vector.reciprocal(rc_solo[:], den_solo[:])
                nc.gpsimd.dma_start(
                    out=rcrow[0:1, h * nq : (h + 1) * nq], in_=rc_solo[:]
                )

            # ---------- prologue ----------
            xq_chunks[0] = xq_chunks0
            kproj_ft(0, 0)
            qproj_ft(0, 0)

            # ---------- fillers: (deadline_slot, order, closure) ----------
            # A filler MUST be emitted before the slot whose instructions
            # consume its output: the Tile framework cannot make a consumer
            # wait on a writer that is emitted later in program order.
            seq = [0]

            def mk(deadline, fn):
                seq[0] += 1
                return (deadline, seq[0], fn)

            def make_fillers(qc):
                f = []
                if qc == 0:
                    # head-0 pair t's attnV (emitted at slot t+1) reads vp
                    # tiles 2t, 2t+1
                    for st in range(nkt):
                        f.append(mk(max(0, st // 2 - 1), lambda st=st: vproj_st(st)))
                    # remaining K-proj sc chunks of ft0: pair 2sc reads them
                    for sc in range(1, 4):
                        f.append(mk(2 * sc - 1, lambda sc=sc: kproj_ft(0, sc)))
                    # K/Q feature tile ft first consumed by head 2*ft at
                    # slot 16*ft; spread deadlines over the preceding head
                    for ft in range(1, nft):
                        base = 16 * ft
                        for sc in range(4):
                            f.append(
                                mk(base - 6 + sc,
                                   lambda ft=ft, sc=sc: kproj_ft(ft, sc))
                            )
                        f.append(mk(base - 1, lambda ft=ft: qproj_ft(0, ft)))
                    for ft in range(nft):
                        f.append(mk(nslots, lambda ft=ft: qproj_ft(1, ft)))
                else:
                    # previous chunk's normalize, then its O-projection
                    for h in range(hpc):
                        dl = 3 + h if h < 4 else 8 + h
                        f.append(mk(dl, lambda q=qc - 1, h=h: normalize_head(q, h)))
                    for jt in range(d // P):
                        f.append(mk(nslots, lambda q=qc - 1, j=jt: oproj_jt(q, j)))
                        if qc < nqc - 1 and jt % 2 == 0:
                            f.append(
                                mk(nslots,
                                   lambda q=qc + 1, ft=jt // 2: qproj_ft(q, ft))
                            )
                return collections.deque(sorted(f, key=lambda x: (x[0], x[1])))

            # ---------- main attention loop ----------
            for qc in range(nqc):
                qsl = slice(qc * nq, (qc + 1) * nq)
                fillers = make_fillers(qc)
                nfill = len(fillers)
                for h in range(hpc):
                    tp = h // 2
                    hr = slice((h % 2) * 64, (h % 2) * 64 + 64)
                    hp = (h % 2) * 64
                    u = ps.tile([P, nq], F32, tag="u", bufs=1)

                    def attnv(pt, pet, h=h, u=u):
                        for j in range(2):
                            nc.tensor.matmul(
                                u[:, :],
                                lhsT=vp_t[:, 2 * pt + j, h, :],
                                rhs=pet[:, j, :],
                                start=(pt == 0 and j == 0),
                                stop=(pt == npair - 1 and j == 1),
                            )

                    prev = None
                    for t in range(npair):
                        slot = h * npair + t
                        pp2 = ps.tile([P, 2, nq], F32, tag="pp", bufs=2)
                        for j in range(2):
                            kt = 2 * t + j
                            nc.tensor.matmul(
                                pp2[:, j, :],
                                lhsT=kt_t[hr, tp, kt * P : (kt + 1) * P],
                                rhs=qt_t[hr, tp, qsl],
                                start=True,
                                stop=True,
                            )
                        et = sb.tile([P, 2, nq], BF16, tag="et", bufs=3)
                        nc.scalar.activation(
                            et[:, :, :],
                            pp2[:, :, :],
                            mybir.ActivationFunctionType.Exp,
                            scale=inv_sqrt_dk,
                            bias=biasA[:],
                        )
                        # drain fillers: first any whose deadline arrived,
                        # then spread the rest evenly over the slots
                        while fillers and fillers[0][0] <= slot:
                            fillers.popleft()[2]()
                        want = (nfill * (slot + 1)) // nslots
                        while nfill - len(fillers) < want and fillers:
                            fillers.popleft()[2]()
                        if prev is not None:
                            attnv(*prev)
                        prev = (t, et)
                    attnv(*prev)
                    # denominator -> staging row (partition 64), DMA-restack
                    # into den8[h]; evict unnormalized attn rows (partition
                    # shift +64 for odd heads is 32-aligned, legal)
                    dstage = sb.tile([65, nq], BF16, tag="dstage", bufs=2)
                    nc.vector.tensor_copy(dstage[64:65, :], u[64:65, :])
                    if qc == nqc - 1:
                        recip_head(h, dstage)
                    else:
                        dh = den_a[h : h + 1, :] if h < 4 else den_b[h - 4 : h - 3, :]
                        nc.sync.dma_start(out=dh, in_=dstage[64:65, :])
                        if h == 3 or h == 7:
                            dn, rc = (den_a, rc_a) if h == 3 else (den_b, rc_b)
                            with nc.allow_low_precision(
                                reason="softmax denominator recip"
                            ):
                                nc.vector.reciprocal(rc[:], dn[:])
                    nc.vector.tensor_copy(attnT[hp : hp + 64, tp, qsl], u[0:64, :])
                while fillers:
                    fillers.popleft()[2]()

            # ---------- epilogue: last chunk's normalize + O-projection ----
            # (recips + rcrow DMAs were emitted per-head during the chunk)
            for h in range(hpc):
                normalize_head(nqc - 1, h, dma=False)
            for jt in range(d // P):
                oproj_jt(nqc - 1, jt)

    nc.compile()
    return nc


def _get_nc(bias):
    if bias not in _NC_CACHE:
        _NC_CACHE[bias] = build_nc(bias=bias)
    return _NC_CACHE[bias]


def make_in_maps(query, key_, value, w_q, b_q, w_k, b_k, w_v, b_v, w_o, b_o):
    bias = bool(np.any(b_q) or np.any(b_k) or np.any(b_v))
    bf = ml_dtypes.bfloat16
    xT = {}
    for b in range(B):
        xT[("q", b)] = np.ascontiguousarray(query[b].T).astype(bf)
        xT[("k", b)] = np.ascontiguousarray(key_[b].T).astype(bf)
        xT[("v", b)] = np.ascontiguousarray(value[b].T).astype(bf)
    wT = {}
    for g in range(GROUPS):
        rows = slice(g * FC, (g + 1) * FC)
        wT[("q", g)] = np.ascontiguousarray(w_q[rows, :].T).astype(bf)
        wT[("k", g)] = np.ascontiguousarray(w_k[rows, :].T).astype(bf)
        wT[("v", g)] = np.ascontiguousarray(w_v[rows, :].T).astype(bf)
        wT[("o", g)] = np.ascontiguousarray(w_o[:, rows].T).astype(bf)
    in_maps = []
    for core in range(NCORES):
        b, g = core // GROUPS, core % GROUPS
        m = {
            "xqT": xT[("q", b)],
            "xkT": xT[("k", b)],
            "xvT": xT[("v", b)],
            "wqT": wT[("q", g)],
            "wkT": wT[("k", g)],
            "wvT": wT[("v", g)],
            "woT": wT[("o", g)],
        }
        if bias:
            rows = slice(g * FC, (g + 1) * FC)
            m["bq"] = np.ascontiguousarray(b_q[rows]).reshape(1, FC).astype(bf)
            m["bk"] = np.ascontiguousarray(b_k[rows]).reshape(1, FC).astype(bf)
            m["bv"] = np.ascontiguousarray(b_v[rows]).reshape(1, FC).astype(bf)
        in_maps.append(m)
    return in_maps, bias


def assemble(results, b_o):
    out = np.empty((B, S, D), np.float32)
    for b in range(B):
        acc = results[b * GROUPS]["outT"].copy()
        for g in range(1, GROUPS):
            acc += results[b * GROUPS + g]["outT"]
        out[b] = acc.T
    out += np.asarray(b_o, np.float32)
    return out


def kernel(
    query,
    key_,
    value,
    w_q,
    b_q,
    w_k,
    b_k,
    w_v,
    b_v,
    w_o,
    b_o,
):
    args = [
        np.asarray(a, np.float32)
        for a in (query, key_, value, w_q, b_q, w_k, b_k, w_v, b_v, w_o, b_o)
    ]
    query, key_, value, w_q, b_q, w_k, b_k, w_v, b_v, w_o, b_o = args
    in_maps, bias = make_in_maps(
        query, key_, value, w_q, b_q, w_k, b_k, w_v, b_v, w_o, b_o
    )
    nc = _get_nc(bias)
    from concourse.bass_utils import run_bass_kernel_spmd

    res = run_bass_kernel_spmd(nc, in_maps, list(range(NCORES)))
    return assemble(res.results, b_o)


# revision 24
# speedup vs baseline: 1.0672x; 1.0672x over previous
"""Multi-head attention (B=4, S=2048, D=1024, H=16) on 8 Trainium2 cores.

Sharding: core = (batch b, head-group g): 4 batches x 2 groups of 8 heads.

Per core (feature-major layouts):
  KT/QT = (x @ W^T)^T        [512, S] bf16
  V'    = x @ Wv^T           [S, 8 heads x (64 V cols + ones col)] bf16
  attention in q-chunks of 1024 (bf16 moving-operand max), per head:
    per kt tile: scores^T[k, q] = 1 MM (contraction 64, N=1024),
    exp on ACT (scale 1/8, bias -2) -> et bf16 [128, 1024],
    attnV MM (M=65, N=1024) accumulating u[0:65]; row 64 = softmax
    denominator via the V' ones column.
  denominators DMA-restacked into [8, 1024]; one batched DVE reciprocal
  per q-chunk; per-head broadcast via ones-matmul; in-place normalize of
  attnT; O-projection (bf16) -> outT partials.
Host: per batch, sum the two groups' outT partials, transpose, add b_o.

Softmax skips max-subtraction (scores ~N(0,1)); exp biased by -2
(shift-invariant, keeps exp outputs in a comfortable range).

The whole kernel is software-pipelined around the ACT-bound attention
loop: K/V/Q/O-projection matmuls and the previous chunk's normalize are
emitted as deadline-scheduled fillers inside the attention slots, so the
PE never head-of-line blocks on ACT output.
"""

import collections

import ml_dtypes
import numpy as np

import concourse.bass as bass
import concourse.mybir as mybir
import concourse.tile as tile
from concourse import bacc

B = 4
S = 2048
D = 1024
H = 16
DK = 64
NCORES = 8
GROUPS = 2
HPC = H // GROUPS  # heads per core (8)
FC = HPC * DK  # local features per core (512)
P = 128

F32 = mybir.dt.float32
BF16 = mybir.dt.bfloat16

VW = 128  # per-head V' width: 64 V + ones col + pad (FWL wants 128)
EXP_BIAS = -2.0

_NC_CACHE = {}


def build_nc(bias=False):
    s, d, fc, hpc = S, D, FC, HPC
    ndt = d // P  # 8 contraction tiles for projections
    nft = fc // P  # 4 local feature tiles
    nq = 512  # attention q-chunk width
    nqc = s // nq  # 4 q chunks
    nkt = s // P  # 16 k tiles
    npair = nkt // 2  # 8 kt-pair slots per head
    inv_sqrt_dk = 1.0 / float(np.sqrt(DK))
    nslots = hpc * npair  # 64 (head, kt-pair) slots per q-chunk

    nc = bacc.Bacc("TRN2", target_bir_lowering=False, debug=False)

    xqT = nc.dram_tensor("xqT", [d, s], BF16, kind="ExternalInput").ap()
    xkT = nc.dram_tensor("xkT", [d, s], BF16, kind="ExternalInput").ap()
    xvT = nc.dram_tensor("xvT", [d, s], BF16, kind="ExternalInput").ap()
    wqT = nc.dram_tensor("wqT", [d, fc], BF16, kind="ExternalInput").ap()
    wkT = nc.dram_tensor("wkT", [d, fc], BF16, kind="ExternalInput").ap()
    wvT = nc.dram_tensor("wvT", [d, fc], BF16, kind="ExternalInput").ap()
    woT = nc.dram_tensor("woT", [fc, d], BF16, kind="ExternalInput").ap()
    outT = nc.dram_tensor("outT", [d, s], F32, kind="ExternalOutput").ap()
    if bias:
        bq = nc.dram_tensor("bq", [1, fc], BF16, kind="ExternalInput").ap()
        bk = nc.dram_tensor("bk", [1, fc], BF16, kind="ExternalInput").ap()
        bv = nc.dram_tensor("bv", [1, fc], BF16, kind="ExternalInput").ap()

    ones8_dram = nc.inline_tensor(
        np.ones((8, 64), ml_dtypes.bfloat16), name="ones8_c"
    ).ap()
    onesv_dram = nc.inline_tensor(
        np.ones((P, hpc), ml_dtypes.bfloat16), name="onesv_c"
    ).ap()
    biasA_dram = nc.inline_tensor(
        np.full((P, 1), EXP_BIAS, np.float32), name="biasA_c"
    ).ap()
    ones1_dram = nc.inline_tensor(
        np.ones((1, 512), ml_dtypes.bfloat16), name="ones1_c"
    ).ap()

    with tile.TileContext(nc) as tc:
        with (
            tc.tile_pool(name="sb", bufs=1) as sb,
            tc.tile_pool(name="ps", bufs=1, space="PSUM") as ps,
        ):
            kt_t = sb.tile([P, nft, s], BF16, tag="KT")
            qt_t = sb.tile([P, nft, s], BF16, tag="QT")
            attnT = qt_t  # attnT(h, qc) overwrites QT columns already consumed
            xk_t = sb.tile([P, ndt, s], BF16, tag="xk")
            xv_t = sb.tile([P, ndt, s], BF16, tag="xv")
            vp_t = sb.tile([P, nkt, hpc, VW], BF16, tag="Vp")
            wk_t = sb.tile([P, ndt, fc], BF16, tag="wk")
            wq_t = sb.tile([P, ndt, fc], BF16, tag="wq")
            wv_t = sb.tile([P, ndt, fc], BF16, tag="wv")
            wo_t = sb.tile([P, fc // P, d], BF16, tag="wo")
            den_a = sb.tile([4, nq], BF16, tag="den_a")
            den_b = sb.tile([4, nq], BF16, tag="den_b")
            rc_a = sb.tile([4, nq], BF16, tag="rc_a")
            rc_b = sb.tile([4, nq], BF16, tag="rc_b")
            rcrow = sb.tile([1, hpc * nq], BF16, tag="rcrow")
            ones8 = sb.tile([8, 64], BF16, tag="ones8")
            onesv = sb.tile([P, hpc], BF16, tag="onesv")
            biasA = sb.tile([P, 1], F32, tag="biasA")
            warm = sb.tile([8, 64], BF16, tag="warm")

            def dma_split(dst, src_ap, n):
                # split big loads across DMA queues; alternate issuing engine
                for i in range(n):
                    eng = nc.sync if i % 2 == 0 else nc.gpsimd
                    eng.dma_start(out=dst[:, i], in_=src_ap[:, i])

            nc.sync.dma_start(out=ones8[:], in_=ones8_dram)
            nc.gpsimd.dma_start(out=onesv[:], in_=onesv_dram)
            nc.sync.dma_start(out=biasA[:], in_=biasA_dram)
            if bias:
                ones1 = sb.tile([1, 512], BF16, tag="ones1")
                nc.sync.dma_start(out=ones1[:], in_=ones1_dram)
                bq_t = sb.tile([1, fc], BF16, tag="bq")
                bk_t = sb.tile([1, fc], BF16, tag="bk")
                bv_t = sb.tile([1, fc], BF16, tag="bv")
                nc.sync.dma_start(out=bq_t[:], in_=bq)
                nc.sync.dma_start(out=bk_t[:], in_=bk)
                nc.sync.dma_start(out=bv_t[:], in_=bv)

            # warm up the ACT exp table during the initial DMA wait
            nc.scalar.activation(
                warm[:],
                ones8[:],
                mybir.ActivationFunctionType.Exp,
                scale=inv_sqrt_dk,
                bias=biasA[0:8, :],
            )

            # weights/x loads with COARSE DMAs (each dma_start costs ~600ns
            # of issuing-engine time; keep the instruction count low).
            # First-score gate: wk + xk(sc0) + wq + xq0.
            wk_r = wkT.rearrange("(t p) f -> p t f", p=P)
            wq_r = wqT.rearrange("(t p) f -> p t f", p=P)
            xk_r = xkT.rearrange("(t p) (c n) -> p t c n", p=P, n=512)
            x0_t = sb.tile([P, ndt, nq], BF16, tag="xq", bufs=1)
            xq_r0 = xqT[:, 0:nq].rearrange("(t p) s -> p t s", p=P)
            # gate: wk+xk(sc0) for K-proj, wq+x0 for Q-proj. ~1.3MB/queue.
            nc.sync.dma_start(out=wk_t[:, 0:4], in_=wk_r[:, 0:4])
            nc.gpsimd.dma_start(out=wk_t[:, 4:8], in_=wk_r[:, 4:8])
            nc.scalar.dma_start(out=wq_t[:, 0:8], in_=wq_r[:, 0:8])
            nc.sync.dma_start(out=xk_t[:, 0:4, 0:512], in_=xk_r[:, 0:4, 0])
            nc.gpsimd.dma_start(out=xk_t[:, 4:8, 0:512], in_=xk_r[:, 4:8, 0])
            nc.scalar.dma_start(out=x0_t[:, 0:3], in_=xq_r0[:, 0:3])
            nc.sync.dma_start(out=x0_t[:, 3:6], in_=xq_r0[:, 3:6])
            nc.gpsimd.dma_start(out=x0_t[:, 6:8], in_=xq_r0[:, 6:8])
            xq_chunks0 = x0_t
            # V-side: resident xv + wv
            wv_r = wvT.rearrange("(t p) f -> p t f", p=P)
            nc.scalar.dma_start(out=wv_t[:, 0:4], in_=wv_r[:, 0:4])
            nc.scalar.dma_start(out=wv_t[:, 4:8], in_=wv_r[:, 4:8])
            xv_r = xvT.rearrange("(t p) s -> p t s", p=P)
            for i in range(4):
                eng = nc.sync if i % 2 == 0 else nc.gpsimd
                eng.dma_start(
                    out=xv_t[:, 2 * i : 2 * i + 2, 0:1024],
                    in_=xv_r[:, 2 * i : 2 * i + 2, 0:1024],
                )
            # rest of xk (sc1 first: needed by pair 2), interleaved with xv
            xk_full = xkT.rearrange("(t p) s -> p t s", p=P)
            for i in range(4):
                eng = nc.sync if i % 2 == 0 else nc.gpsimd
                eng.dma_start(
                    out=xk_t[:, 2 * i : 2 * i + 2, 512:1024],
                    in_=xk_full[:, 2 * i : 2 * i + 2, 512:1024],
                )
            for i in range(4):
                eng = nc.gpsimd if i % 2 == 0 else nc.sync
                eng.dma_start(
                    out=xv_t[:, 2 * i : 2 * i + 2, 1024:2048],
                    in_=xv_r[:, 2 * i : 2 * i + 2, 1024:2048],
                )
            for i in range(4):
                eng = nc.sync if i % 2 == 0 else nc.gpsimd
                eng.dma_start(
                    out=xk_t[:, 2 * i : 2 * i + 2, 1024:2048],
                    in_=xk_full[:, 2 * i : 2 * i + 2, 1024:2048],
                )
            wo_r = woT.rearrange("(t p) j -> p t j", p=P)
            nc.scalar.dma_start(out=wo_t[:, 0:2], in_=wo_r[:, 0:2])
            nc.scalar.dma_start(out=wo_t[:, 2:4], in_=wo_r[:, 2:4])

            # ---------- emission helpers ----------
            def kproj_ft(ft, sc):
                acc = ps.tile([P, 512], F32, tag="acc", bufs=2)
                first = True
                if bias:
                    nc.tensor.matmul(
                        acc[:],
                        lhsT=bk_t[0:1, ft * P : (ft + 1) * P],
                        rhs=ones1[0:1, :],
                        start=True,
                        stop=False,
                    )
                    first = False
                for dt in range(ndt):
                    nc.tensor.matmul(
                        acc[:],
                        lhsT=wk_t[:, dt, ft * P : (ft + 1) * P],
                        rhs=xk_t[:, dt, sc * 512 : (sc + 1) * 512],
                        start=(dt == 0 and first),
                        stop=(dt == ndt - 1),
                    )
                nc.vector.tensor_copy(kt_t[:, ft, sc * 512 : (sc + 1) * 512], acc[:])

            xq_chunks = {}

            def qproj_ft(qc, ft):
                qsl = slice(qc * nq, (qc + 1) * nq)
                if qc not in xq_chunks:
                    x_t = sb.tile([P, ndt, nq], BF16, tag="xq", bufs=1)
                    xq_r = xqT[:, qsl].rearrange("(t p) s -> p t s", p=P)
                    nc.sync.dma_start(out=x_t[:, 0:4], in_=xq_r[:, 0:4])
                    nc.gpsimd.dma_start(out=x_t[:, 4:8], in_=xq_r[:, 4:8])
                    xq_chunks[qc] = x_t
                x_t = xq_chunks[qc]
                acc = ps.tile([P, 512], F32, tag="acc", bufs=2)
                first = True
                if bias:
                    nc.tensor.matmul(
                        acc[:],
                        lhsT=bq_t[0:1, ft * P : (ft + 1) * P],
                        rhs=ones1[0:1, :],
                        start=True,
                        stop=False,
                    )
                    first = False
                for dt in range(ndt):
                    nc.tensor.matmul(
                        acc[:],
                        lhsT=wq_t[:, dt, ft * P : (ft + 1) * P],
                        rhs=x_t[:, dt, :],
                        start=(dt == 0 and first),
                        stop=(dt == ndt - 1),
                    )
                nc.vector.tensor_copy(qt_t[:, ft, qsl], acc[:])

            def vproj_st(st):
                acc = ps.tile([P, hpc, DK], F32, tag="acc", bufs=2)
                first = True
                if bias:
                    nc.tensor.matmul(
                        acc[:, :, :],
                        lhsT=ones1[0:1, 0:P],
                        rhs=bv_t[0:1, :],
                        start=True,
                        stop=False,
                    )
                    first = False
                for dt in range(ndt):
                    nc.tensor.matmul(
                        acc[:, :, :],
                        lhsT=xv_t[:, dt, st * P : (st + 1) * P],
                        rhs=wv_t[:, dt, :],
                        start=(dt == 0 and first),
                        stop=(dt == ndt - 1),
                    )
                nc.vector.tensor_copy(vp_t[:, st, :, 0:DK], acc[:])
                nc.vector.tensor_copy(vp_t[:, st, :, DK], onesv[:])

            ot_pair = {}

            def oproj_jt(qc, jt):
                qsl = slice(qc * nq, (qc + 1) * nq)
                acc = ps.tile([P, 512], F32, tag="acc", bufs=2)
                for ct in range(fc // P):
                    nc.tensor.matmul(
                        acc[:],
                        lhsT=wo_t[:, ct, jt * P : (jt + 1) * P],
                        rhs=attnT[:, ct, qsl],
                        start=(ct == 0),
                        stop=(ct == fc // P - 1),
                    )
                # pair two jt evictions into one store DMA
                if jt % 2 == 0:
                    ot_pair[qc] = sb.tile([P, 2, 512], F32, tag="ot", bufs=2, name="ot2")
                ot = ot_pair[qc]
                nc.vector.tensor_copy(ot[:, jt % 2, :], acc[:])
                if jt % 2 == 1:
                    nc.gpsimd.dma_start(
                        out=outT[(jt - 1) * P : (jt + 1) * P, qsl].rearrange(
                            "(j p) n -> p j n", p=P
                        ),
                        in_=ot[:],
                    )

            def normalize_head(qc, h, dma=True):
                # emitted AFTER the recip covering head h is emitted
                qsl = slice(qc * nq, (qc + 1) * nq)
                tp = h // 2
                hp = (h % 2) * 64
                hsl = slice(hp, hp + 64)
                if dma:
                    rc = rc_a[h : h + 1, :] if h < 4 else rc_b[h - 4 : h - 3, :]
                    nc.gpsimd.dma_start(
                        out=rcrow[0:1, h * nq : (h + 1) * nq], in_=rc
                    )
                pbx = ps.tile([P, nq], F32, tag="acc", bufs=2, name="pbx")
                nc.tensor.matmul(
                    pbx[hsl, :],
                    lhsT=ones8[0:1, :],
                    rhs=rcrow[0:1, h * nq : (h + 1) * nq],
                    start=True,
                    stop=True,
                )
                nc.vector.tensor_mul(
                    attnT[hsl, tp, qsl], attnT[hsl, tp, qsl], pbx[hsl, :]
                )

            def recip_head(h, dstage):
                # per-head recip for the last chunk (overlaps attention
                # instead of idling the PE tail). Partition-0 tiles only:
                # unaligned partition starts are illegal on the DVE.
                den_solo = sb.tile([1, nq], BF16, tag="dsolo", bufs=2)
                nc.gpsimd.dma_start(out=den_solo[:], in_=dstage[64:65, :])
                rc_solo = sb.tile([1, nq], BF16, tag="rsolo", bufs=2)
                with nc.allow_low_precision(reason="softmax denominator recip"):
                    nc.vector.reciprocal(rc_solo[:], den_solo[:])
                nc.gpsimd.dma_start(
                    out=rcrow[0:1, h * nq : (h + 1) * nq], in_=rc_solo[:]
                )

            # ---------- prologue ----------
            xq_chunks[0] = xq_chunks0
            kproj_ft(0, 0)
            qproj_ft(0, 0)

            # ---------- fillers: (deadline_slot, order, closure) ----------
            # A filler MUST be emitted before the slot whose instructions
            # consume its output: the Tile framework cannot make a consumer
            # wait on a writer that is emitted later in program order.
            seq = [0]

            def mk(deadline, fn):
                seq[0] += 1
                return (deadline, seq[0], fn)

            def make_fillers(qc):
                f = []
                if qc == 0:
                    # head-pair 0's attnV at slot kt+1 reads vp tile kt
                    for st in range(nkt):
                        f.append(mk(max(0, st - 1), lambda st=st: vproj_st(st)))
                    # remaining K-proj sc chunks of ft0: kt tile 4sc first
                    # read at slot 4sc
                    for sc in range(1, 4):
                        f.append(mk(4 * sc - 2, lambda sc=sc: kproj_ft(0, sc)))
                    # K/Q feature tile ft first consumed by pair ft at
                    # slot 16*ft; spread deadlines over the preceding pair
                    for ft in range(1, nft):
                        base = 16 * ft
                        for sc in range(4):
                            f.append(
                                mk(base - 6 + sc,
                                   lambda ft=ft, sc=sc: kproj_ft(ft, sc))
                            )
                        f.append(mk(base - 1, lambda ft=ft: qproj_ft(0, ft)))
                    for ft in range(nft):
                        f.append(mk(nslots, lambda ft=ft: qproj_ft(1, ft)))
                else:
                    # previous chunk's normalize, then its O-projection
                    for h in range(hpc):
                        dl = 3 + h if h < 4 else 8 + h
                        f.append(mk(dl, lambda q=qc - 1, h=h: normalize_head(q, h)))
                    for jt in range(d // P):
                        f.append(mk(nslots, lambda q=qc - 1, j=jt: oproj_jt(q, j)))
                        if qc < nqc - 1 and jt % 2 == 0:
                            f.append(
                                mk(nslots,
                                   lambda q=qc + 1, ft=jt // 2: qproj_ft(q, ft))
                            )
                return collections.deque(sorted(f, key=lambda x: (x[0], x[1])))

            # ---------- main attention loop ----------
            # heads are processed in PAIRS (2i, 2i+1) sharing feature tile
            # tp=i: their score matmuls use disjoint PE row groups (rows
            # 0:64 vs 64:128) and run concurrently in the array.
            for qc in range(nqc):
                qsl = slice(qc * nq, (qc + 1) * nq)
                fillers = make_fillers(qc)
                nfill = len(fillers)
                for i in range(hpc // 2):
                    tp = i
                    u_a = ps.tile([P, nq], F32, tag="u", bufs=2, name="u_a")
                    u_b = ps.tile([P, nq], F32, tag="u", bufs=2, name="u_b")

                    def attnv(pkt, pet, i=i, u_a=u_a, u_b=u_b):
                        for j, u in ((0, u_a), (1, u_b)):
                            nc.tensor.matmul(
                                u[:, :],
                                lhsT=vp_t[:, pkt, 2 * i + j, :],
                                rhs=pet[:, j, :],
                                start=(pkt == 0),
                                stop=(pkt == nkt - 1),
                            )

                    prev = None
                    for kt in range(nkt):
                        slot = i * nkt + kt
                        pp = ps.tile([P, 2, nq], F32, tag="pp", bufs=2)
                        for j in range(2):
                            hr = slice(j * 64, j * 64 + 64)
                            nc.tensor.matmul(
                                pp[:, j, :],
                                lhsT=kt_t[hr, tp, kt * P : (kt + 1) * P],
                                rhs=qt_t[hr, tp, qsl],
                                start=True,
                                stop=True,
                            )
                        et = sb.tile([P, 2, nq], BF16, tag="et", bufs=3)
                        nc.scalar.activation(
                            et[:, :, :],
                            pp[:, :, :],
                            mybir.ActivationFunctionType.Exp,
                            scale=inv_sqrt_dk,
                            bias=biasA[:],
                        )
                        # drain fillers: deadline-due first, then spread
                        while fillers and fillers[0][0] <= slot:
                            fillers.popleft()[2]()
                        want = (nfill * (slot + 1)) // nslots
                        while nfill - len(fillers) < want and fillers:
                            fillers.popleft()[2]()
                        if prev is not None:
                            attnv(*prev)
                        prev = (kt, et)
                    attnv(*prev)
                    for j, u in ((0, u_a), (1, u_b)):
                        h = 2 * i + j
                        hp = (h % 2) * 64
                        dstage = sb.tile([65, nq], BF16, tag="dstage", bufs=2)
                        nc.vector.tensor_copy(dstage[64:65, :], u[64:65, :])
                        if qc == nqc - 1:
                            recip_head(h, dstage)
                        else:
                            dh = (
                                den_a[h : h + 1, :]
                                if h < 4
                                else den_b[h - 4 : h - 3, :]
                            )
                            nc.sync.dma_start(out=dh, in_=dstage[64:65, :])
                            if h == 3 or h == 7:
                                dn, rc = (den_a, rc_a) if h == 3 else (den_b, rc_b)
                                with nc.allow_low_precision(
                                    reason="softmax denominator recip"
                                ):
                                    nc.vector.reciprocal(rc[:], dn[:])
                        nc.vector.tensor_copy(attnT[hp : hp + 64, tp, qsl], u[0:64, :])
                while fillers:
                    fillers.popleft()[2]()

            # ---------- epilogue: last chunk's normalize + O-projection ----
            # (recips + rcrow DMAs were emitted per-head during the chunk)
            for h in range(hpc):
                normalize_head(nqc - 1, h, dma=False)
            for jt in range(d // P):
                oproj_jt(nqc - 1, jt)

    nc.compile()
    return nc


def _get_nc(bias):
    if bias not in _NC_CACHE:
        _NC_CACHE[bias] = build_nc(bias=bias)
    return _NC_CACHE[bias]


def make_in_maps(query, key_, value, w_q, b_q, w_k, b_k, w_v, b_v, w_o, b_o):
    bias = bool(np.any(b_q) or np.any(b_k) or np.any(b_v))
    bf = ml_dtypes.bfloat16
    xT = {}
    for b in range(B):
        xT[("q", b)] = np.ascontiguousarray(query[b].T).astype(bf)
        xT[("k", b)] = np.ascontiguousarray(key_[b].T).astype(bf)
        xT[("v", b)] = np.ascontiguousarray(value[b].T).astype(bf)
    wT = {}
    for g in range(GROUPS):
        rows = slice(g * FC, (g + 1) * FC)
        wT[("q", g)] = np.ascontiguousarray(w_q[rows, :].T).astype(bf)
        wT[("k", g)] = np.ascontiguousarray(w_k[rows, :].T).astype(bf)
        wT[("v", g)] = np.ascontiguousarray(w_v[rows, :].T).astype(bf)
        wT[("o", g)] = np.ascontiguousarray(w_o[:, rows].T).astype(bf)
    in_maps = []
    for core in range(NCORES):
        b, g = core // GROUPS, core % GROUPS
        m = {
            "xqT": xT[("q", b)],
            "xkT": xT[("k", b)],
            "xvT": xT[("v", b)],
            "wqT": wT[("q", g)],
            "wkT": wT[("k", g)],
            "wvT": wT[("v", g)],
            "woT": wT[("o", g)],
        }
        if bias:
            rows = slice(g * FC, (g + 1) * FC)
            m["bq"] = np.ascontiguousarray(b_q[rows]).reshape(1, FC).astype(bf)
            m["bk"] = np.ascontiguousarray(b_k[rows]).reshape(1, FC).astype(bf)
            m["bv"] = np.ascontiguousarray(b_v[rows]).reshape(1, FC).astype(bf)
        in_maps.append(m)
    return in_maps, bias


def assemble(results, b_o):
    out = np.empty((B, S, D), np.float32)
    for b in range(B):
        acc = results[b * GROUPS]["outT"].copy()
        for g in range(1, GROUPS):
            acc += results[b * GROUPS + g]["outT"]
        out[b] = acc.T
    out += np.asarray(b_o, np.float32)
    return out


def kernel(
    query,
    key_,
    value,
    w_q,
    b_q,
    w_k,
    b_k,
    w_v,
    b_v,
    w_o,
    b_o,
):
    args = [
        np.asarray(a, np.float32)
        for a in (query, key_, value, w_q, b_q, w_k, b_k, w_v, b_v, w_o, b_o)
    ]
    query, key_, value, w_q, b_q, w_k, b_k, w_v, b_v, w_o, b_o = args
    in_maps, bias = make_in_maps(
        query, key_, value, w_q, b_q, w_k, b_k, w_v, b_v, w_o, b_o
    )
    nc = _get_nc(bias)
    from concourse.bass_utils import run_bass_kernel_spmd

    res = run_bass_kernel_spmd(nc, in_maps, list(range(NCORES)))
    return assemble(res.results, b_o)
